# revision 57
# baseline (speedup 1.0000x reference)
"""LoRA Linear (x @ W.T + b + scaling * (x @ A.T) @ B.T) on 8 TRN2 NeuronCores.

Strategy (all-fp8 DoubleRow: base, adapter projection, and close):
  - Data-parallel: 8192 tokens -> 8 x 1024 rows, one shard per core.
  - Base matmul in fp8 e4m3 with MatmulPerfMode.DoubleRow (2 k-rows per
    partition, 2 moving rows/cycle -> 256 cycles per [128tok x 512out x 256k]
    instruction). 109.2us/core is the PE floor at 2.4 GHz.
  - x shipped as xf8 = fp8(8x) plus residual xr8 = fp8(512(x-xf8/8)); the
    base path uses xf8 alone (~1.4% rel err, dominated by fp8 noise on x
    and W); the adapter path uses both.
  - Adapter projection (per m-tile, all fp8 DoubleRow, two accumulators so
    no operand goes subnormal): psA = 8x (.) 4(A_hi+A_lo) = 32*x8@A over
    k-tile pairs; psB = 512xr (.) 4A_hi = 2048*xr@A_hi; combined on DVE as
    xa_c = psB/64 + psA = 32*xa (the dropped xr@A_lo term is ~0.2% of xa).
    Then an on-device fp8 hi/lo split (hi8 = fp8(xa_c/32), lo = xa - hi8),
    packed [128, 32], one PE transpose, copied twice into
    xaT8[33, 2, 1024] (partitions 0-15 hi rows, 16-31 lo rows, 32 = ones
    for the bias; t-dim duplicated).
  - Close: ONE fp8 DoubleRow matmul per group finishes each PSUM
    accumulation: psum += xaT8.T @ bb8 where bb8 pairs (B_hi, B_lo) per
    partition => (xa_hi+xa_lo)(B_hi+B_lo) = 2048*2*(xa@B.T), and partition
    32 adds 2048*b via its (b_hi, b_lo) pair. 256 cycles instead of the
    bf16 close's 512.
  - Scales: PSC = SX*SW = 8*256 = 2048 so bb8 = fp8(4096*B.T) stays under
    fp8e4 max 240. Host divides the output by 2048 (power of two, exact).
  - PE p-state (cost model): clock reaches 2.4 GHz only after ~3us of
    sustained execution and a >4us idle gap resets it -> 30 warmup dummies
    inside the initial DMA window, then no PE gap exceeds ~1.5us.
  - DMA: all inputs on the SP HWDGE queue, ordered so cumulative delivered
    bytes track PE consumption (x0/W0-quarters/x1 first, then x_i
    interleaved with w1..w3 halves, w4..w7 last, gated by the W pool's 5
    bufs). Outputs are buffered in SBUF and flushed one whole column per
    completion on the Activation queue - they never delay an input the PE
    is waiting for. Last column goes out per-m-tile on the by-then-idle SP
    queue to shorten the tail.
  - PE order: piecewise opens of (tiles 0-2, col0) as W0 quarters land with
    xa/transposes as filler, cols 1-2 for tiles 0-2 as w1/w2 halves land,
    then per-tile bands (i=3..7 across cols 0-2), then cols 3-7
    column-major. The cols 1-2 opens hide the xa->split->transpose->copy
    chain of each tile before its closes read xaT8.
"""

import numpy as np
import ml_dtypes

import concourse.bass as bass
from concourse import bacc
import concourse.mybir as mybir
import concourse.tile as tile
from concourse.bass_utils import run_bass_kernel_spmd

N_CORES = 8
IN_F = 4096
OUT_F = 4096
RANK = 16
ALPHA = 32.0
B_SZ = 4
S_SZ = 2048
TOK = B_SZ * S_SZ            # 8192
M_PER_CORE = TOK // N_CORES  # 1024

P = 128                      # partitions
KT = IN_F // P               # 32 k-tiles
KP = KT // 2                 # 16 k-pair tiles (DoubleRow)
O_BLK = 512                  # o-block width (psum bank = 512 fp32)
N_OBLK = OUT_F // O_BLK      # 8
MT = M_PER_CORE // P         # 8 m-tiles
RP = 2 * RANK + 1            # hi rows + lo rows + ones/bias row = 33

SX = 8.0                     # x fp8 scale
SR = 64.0                    # x residual fp8 scale (matches 32x psum
                             # scale through at8r = fp8(A/2))
SW = 256.0                   # W fp8 scale
PSC = SX * SW                # psum scale 2048
SB = PSC * (ALPHA / RANK)    # B scale in bb8 = 4096
DESCALE = 1.0 / PSC          # applied on host (power of two, exact)

F8 = mybir.dt.float8e4
BF = mybir.dt.bfloat16
F32 = mybir.dt.float32
NP_F8 = ml_dtypes.float8_e4m3
NP_BF = ml_dtypes.bfloat16

LAST_RESULTS = None          # test.py reads exec_time_ns from here
_NC_CACHE = None


def _default_schedule():
    # DMA tokens: ("at",)/("bb",)/("id",) consts; ("f8", i)/("r8", i) x
    # m-tiles; ("w", j, kp_lo, kp_hi) W o-block pieces.
    dma_seq = [
        ("f8", 0, 0, 16), ("f8", 0, 16, 32),
        ("w", 0, 0, 4), ("f8", 1), ("w", 0, 4, 8), ("w", 0, 8, 12),
        ("f8", 2), ("w", 0, 12, 16),
        ("w", 1, 0, 8), ("w", 1, 8, 16),
        ("w", 2, 0, 8), ("w", 2, 8, 16),
        ("f8", 3), ("r8", 3), ("id",), ("at",),
        ("r8", 0, 0, 16), ("r8", 0, 16, 32), ("bb",),
        ("f8", 4), ("r8", 4), ("r8", 1, 0, 16), ("r8", 1, 16, 32),
        ("f8", 5), ("r8", 5), ("r8", 2, 0, 16), ("r8", 2, 16, 32),
        ("w", 3, 0, 4), ("w", 3, 4, 8), ("f8", 6), ("r8", 6),
        ("w", 3, 8, 12), ("w", 3, 12, 16),
        ("f8", 7), ("r8", 7),
        ("w", 4, 0, 16), ("w", 5, 0, 16), ("w", 6, 0, 16), ("w", 7, 0, 16),
    ]
    # PE tokens: ("d", n) warmup dummies; ("go", i, j, kp_lo, kp_hi) partial
    # open (stop=True on kp 15 for decoupled groups); ("gbc", i, j) base-only
    # copy-out; ("ga", i, j) late adapter+bias patch (PE matmul + DVE add
    # into the SBUF column tile); ("gc", i, j) fused close; ("g", i, j)
    # full fused group; ("xa", i); ("tr", i).
    # ~30 dummies accumulate the p-state ramp during the DMA prefill.
    pe_seq = [("d", 30)]
    # phase A: tiles 0-2 over col0 as W0 quarters land, base-only (the
    # adapter inputs r8/at/bb are deferred behind w1/w2 on the wire)
    pe_seq += [("go", 0, 0, 0, 4),
               ("go", 1, 0, 0, 4),
               ("go", 0, 0, 4, 8), ("go", 1, 0, 4, 8),
               ("go", 0, 0, 8, 12), ("go", 1, 0, 8, 12),
               ("go", 2, 0, 0, 12),
               ("go", 0, 0, 12, 16), ("gbc", 0, 0),
               ("go", 1, 0, 12, 16), ("gbc", 1, 0),
               ("go", 2, 0, 12, 16), ("gbc", 2, 0)]
    # phase B: tiles 0-2 over cols 1-2 as w1/w2 halves land, all base-only
    pe_seq += [("go", 0, 1, 0, 8), ("go", 1, 1, 0, 8), ("go", 2, 1, 0, 8),
               ("go", 0, 1, 8, 16), ("gbc", 0, 1),
               ("go", 1, 1, 8, 16), ("gbc", 1, 1),
               ("go", 2, 1, 8, 16), ("gbc", 2, 1),
               ("go", 0, 2, 0, 8), ("go", 1, 2, 0, 8), ("go", 2, 2, 0, 8),
               ("go", 0, 2, 8, 16), ("gbc", 0, 2),
               ("go", 1, 2, 8, 16), ("gbc", 1, 2),
               ("go", 2, 2, 8, 16), ("gbc", 2, 2)]
    # phase C: tiles 3-7, each across cols 0-2 (fused closes); the xa/tr of
    # tiles 0-2 and all nine adapter patches ride the natural slack of these
    # blocks (every chain is hidden behind a 1.71us open)
    pe_seq += [("go", 3, 0, 0, 16), ("xa", 3), ("go", 3, 1, 0, 16),
               ("xa", 0), ("tr", 3), ("go", 3, 2, 0, 16), ("gc", 3, 0),
               ("tr", 0), ("gc", 3, 1), ("gc", 3, 2),
               ("ga", 0, 0), ("ga", 0, 1),
               ("go", 4, 0, 0, 16), ("xa", 4), ("go", 4, 1, 0, 16),
               ("xa", 1), ("tr", 4), ("go", 4, 2, 0, 16), ("gc", 4, 0),
               ("tr", 1), ("gc", 4, 1), ("gc", 4, 2),
               ("ga", 1, 0), ("ga", 1, 1), ("ga", 0, 2),
               ("go", 5, 0, 0, 16), ("xa", 5), ("go", 5, 1, 0, 16),
               ("xa", 2), ("tr", 5), ("go", 5, 2, 0, 16), ("gc", 5, 0),
               ("tr", 2), ("gc", 5, 1), ("gc", 5, 2),
               ("ga", 2, 0), ("ga", 2, 1), ("ga", 1, 2),
               ("go", 6, 0, 0, 16), ("xa", 6), ("go", 6, 1, 0, 16),
               ("tr", 6), ("go", 6, 2, 0, 16), ("gc", 6, 0),
               ("ga", 2, 2), ("gc", 6, 1), ("gc", 6, 2),
               ("go", 7, 0, 0, 16), ("xa", 7), ("go", 7, 1, 0, 16),
               ("tr", 7), ("go", 7, 2, 0, 16), ("gc", 7, 0),
               ("gc", 7, 1), ("gc", 7, 2)]
    # phase D: cols 3..7 column-major
    for j in range(3, N_OBLK):
        for i in range(MT):
            pe_seq.append(("g", i, j))
    return dma_seq, pe_seq


def _build_nc(schedule=None):
    dma_seq, pe_seq = schedule if schedule is not None else _default_schedule()

    nc = bacc.Bacc(None, target_bir_lowering=False)

    xf8_d = nc.dram_tensor("xf8", [MT, P, KT, P], F8, kind="ExternalInput")
    xr8_d = nc.dram_tensor("xr8", [MT, P, KT, P], F8, kind="ExternalInput")
    wt_d = nc.dram_tensor("wt", [N_OBLK, P, KP, 2, O_BLK], F8,
                          kind="ExternalInput")
    at_d = nc.dram_tensor("at", [3, P, KP, 2, RANK], F8,
                          kind="ExternalInput")
    bb_d = nc.dram_tensor("bb", [RP, 2, OUT_F], F8, kind="ExternalInput")
    id_d = nc.dram_tensor("ident", [P, P], BF, kind="ExternalInput")
    # [i][p][j][c] row-major == [1024, 4096] row-major
    out_d = nc.dram_tensor("out", [MT, P, N_OBLK, O_BLK], BF,
                           kind="ExternalOutput")

    with tile.TileContext(nc) as tc:
        with (
            tc.tile_pool(name="xp", bufs=1) as xp,
            tc.tile_pool(name="wp", bufs=5) as wp,
            tc.tile_pool(name="sm", bufs=1) as sm,
            tc.tile_pool(name="hip", bufs=2) as hip,
            tc.tile_pool(name="pkp", bufs=2) as pkp,
            tc.tile_pool(name="outs", bufs=4) as outs,
            tc.tile_pool(name="pmain", bufs=6, space="PSUM") as pmain,
            tc.tile_pool(name="pxa", bufs=1, space="PSUM") as pxa,
            tc.tile_pool(name="ptr", bufs=1, space="PSUM") as ptr,
        ):
            # ---- small constants ----
            at8h_sb = sm.tile([P, KP, 2, RANK], F8, tag="at8h")
            at8l_sb = sm.tile([P, KP, 2, RANK], F8, tag="at8l")
            at8r_sb = sm.tile([P, KP, 2, RANK], F8, tag="at8r")
            bb_sb = sm.tile([RP, 2, OUT_F], F8, tag="bb")
            id_sb = sm.tile([P, P], BF, tag="ident")
            warm = sm.tile([P, P], BF, tag="warm")
            nc.vector.memset(warm[:], 0.0)
            xaT8 = sm.tile([RP, 2, M_PER_CORE], F8, tag="xaT8")
            # partition 32 rows stay (1.0, 1.0): multiplied with bb's
            # (b_hi, b_lo) pair they add the bias. Partitions 0..31 are
            # overwritten column-range by column-range by the per-m-tile
            # transposed copies before any close reads them. (Walrus cannot
            # lower a memset at a nonzero partition offset, so set the whole
            # tile.)
            nc.vector.memset(xaT8[:], 1.0)
            smalls = {"bb": (bb_sb, bb_d), "id": (id_sb, id_d)}

            wps = pmain.tile([P, O_BLK], F32, tag="pm", name="warmps")

            xf8s, xr8s = [], []
            for i in range(MT):
                xf8s.append(xp.tile([P, KT, P], F8, tag=f"xf8_{i}",
                                    name=f"xf8_{i}"))
                xr8s.append(xp.tile([P, KT, P], F8, tag=f"xr8_{i}",
                                    name=f"xr8_{i}"))

            w_tiles = {}
            col_tiles = {}

            def dummies(n):
                for _ in range(n):
                    nc.tensor.matmul(wps[:, 0:P], warm[:], warm[:],
                                     start=True, stop=True)

            def dma_w(j, lo, hi):
                if j not in w_tiles:
                    w_tiles[j] = wp.tile([P, KP, 2, O_BLK], F8, tag="w",
                                         name=f"w{j}")
                t = w_tiles[j]
                if lo == 0 and hi == KP:
                    nc.sync.dma_start(t[:], wt_d[j])
                else:
                    nc.sync.dma_start(t[:, lo:hi, :, :],
                                      wt_d[j, :, lo:hi, :, :])

            xa_ps = {}

            def xa(i):
                # All-fp8 DoubleRow adapter projection, single accumulator
                # (operand scales chosen to dodge fp8 subnormals):
                #   ps = sum 8x*(4A_hi+4A_lo) + sum 64xr*(A/2) = 32 * xa
                ps = pxa.tile([P, RANK], F32, tag="pxa", name=f"pxa{i}")
                for kp in range(KP):
                    xp_sl = xf8s[i][:, 2 * kp:2 * kp + 2, :]
                    nc.tensor.matmul(
                        ps[:], xp_sl, at8h_sb[:, kp, :, :],
                        start=(kp == 0), stop=False,
                        perf_mode=mybir.MatmulPerfMode.DoubleRow,
                    )
                    nc.tensor.matmul(
                        ps[:], xp_sl, at8l_sb[:, kp, :, :],
                        start=False, stop=False,
                        perf_mode=mybir.MatmulPerfMode.DoubleRow,
                    )
                    nc.tensor.matmul(
                        ps[:], xr8s[i][:, 2 * kp:2 * kp + 2, :],
                        at8r_sb[:, kp, :, :],
                        start=False, stop=(kp == KP - 1),
                        perf_mode=mybir.MatmulPerfMode.DoubleRow,
                    )
                # fp8 hi/lo split: hi8 = fp8(xa); lo = xa - hi8 (bf16)
                hi8_t = hip.tile([P, RANK], F8, tag="hi8", name=f"hi8_{i}")
                nc.scalar.activation(hi8_t[:], ps[:],
                                     mybir.ActivationFunctionType.Copy,
                                     scale=1.0 / 32.0)
                pk = pkp.tile([P, 2, RANK], BF, tag="pack", name=f"pack{i}")
                nc.vector.tensor_copy(pk[:, 0, :], hi8_t[:])
                nc.vector.scalar_tensor_tensor(
                    pk[:, 1, :], ps[:], 1.0 / 32.0, pk[:, 0, :],
                    mybir.AluOpType.mult, mybir.AluOpType.subtract,
                )
                xa_ps[i] = pk

            def tr(i):
                pk = xa_ps.pop(i)
                trt = ptr.tile([2 * RANK, P], BF, tag="ptr", name=f"ptr{i}")
                nc.tensor.transpose(trt[:], pk[:], id_sb[:])
                cs = slice(i * P, (i + 1) * P)
                nc.scalar.copy(xaT8[0:2 * RANK, 0, cs], trt[:])
                nc.vector.tensor_copy(xaT8[0:2 * RANK, 1, cs], trt[:])

            gidx = 0
            col_remaining = {j: MT for j in range(N_OBLK)}
            open_ps = {}
            # groups whose base part is closed standalone (adapter patched
            # in later by ("ga", i, j))
            decoupled = {(i, j) for i in range(3) for j in range(3)}

            def group_open(i, j, lo, hi):
                if (i, j) in open_ps:
                    ps = open_ps[(i, j)]
                else:
                    ps = pmain.tile([P, O_BLK], F32, tag="pm",
                                    name=f"pm{i}_{j}")
                    open_ps[(i, j)] = ps
                for kp in range(lo, hi):
                    nc.tensor.matmul(
                        ps[:], xf8s[i][:, 2 * kp:2 * kp + 2, :],
                        w_tiles[j][:, kp, :, :],
                        start=(kp == 0),
                        stop=(kp == KP - 1 and (i, j) in decoupled),
                        perf_mode=mybir.MatmulPerfMode.DoubleRow,
                    )

            def ensure_col(j):
                if j not in col_tiles:
                    col_tiles[j] = outs.tile([P, MT, O_BLK], BF, tag="col",
                                             name=f"col{j}")

            def group_base_copy(i, j):
                nonlocal gidx
                ps = open_ps.pop((i, j))
                ensure_col(j)
                if gidx % 2 == 0:
                    nc.scalar.copy(col_tiles[j][:, i, :], ps[:])
                else:
                    nc.vector.tensor_copy(col_tiles[j][:, i, :], ps[:])
                gidx += 1

            def group_adj(i, j):
                # late adapter+bias patch: adj = xaT8.T @ bb8 in a fresh
                # psum, then col_tile += adj on DVE
                ps = pmain.tile([P, O_BLK], F32, tag="pm",
                                name=f"adj{i}_{j}")
                nc.tensor.matmul(
                    ps[:], xaT8[:, :, i * P:(i + 1) * P],
                    bb_sb[:, :, j * O_BLK:(j + 1) * O_BLK],
                    start=True, stop=True,
                    perf_mode=mybir.MatmulPerfMode.DoubleRow,
                )
                nc.vector.scalar_tensor_tensor(
                    col_tiles[j][:, i, :], ps[:], 1.0,
                    col_tiles[j][:, i, :],
                    mybir.AluOpType.mult, mybir.AluOpType.add,
                )
                col_remaining[j] -= 1

            def group_close(i, j):
                nonlocal gidx
                ps = open_ps.pop((i, j))
                ensure_col(j)
                # fp8 DoubleRow close: (xa_hi+xa_lo)(B_hi+B_lo) + bias row
                nc.tensor.matmul(
                    ps[:], xaT8[:, :, i * P:(i + 1) * P],
                    bb_sb[:, :, j * O_BLK:(j + 1) * O_BLK],
                    start=False, stop=True,
                    perf_mode=mybir.MatmulPerfMode.DoubleRow,
                )
                # Output DMAs are deferred to column completion so they never
                # delay input transfers the PE is still waiting for. Last
                # column goes out per-m-tile in half-width pieces (copies
                # split across ACT/DVE, DMAs on the by-then-idle SP queue) to
                # shorten the end-of-kernel tail.
                if j == N_OBLK - 1:
                    # per-m-tile pieces on the by-then-idle SP queue keep the
                    # end-of-kernel tail short
                    if gidx % 2 == 0:
                        nc.scalar.copy(col_tiles[j][:, i, :], ps[:])
                    else:
                        nc.vector.tensor_copy(col_tiles[j][:, i, :], ps[:])
                    nc.sync.dma_start(
                        out_d[i:i + 1, :, j, :].rearrange("i p c -> p i c"),
                        col_tiles[j][:, i:i + 1, :],
                    )
                else:
                    if gidx % 2 == 0:
                        nc.scalar.copy(col_tiles[j][:, i, :], ps[:])
                    else:
                        nc.vector.tensor_copy(col_tiles[j][:, i, :], ps[:])
                gidx += 1
                col_remaining[j] -= 1
                done = MT - col_remaining[j]
                if j < N_OBLK - 1 and done == MT:
                    nc.scalar.dma_start(
                        out_d[:, :, j, :].rearrange("i p c -> p i c"),
                        col_tiles[j][:],
                    )

            for tok in dma_seq:
                kind = tok[0]
                if kind == "at":
                    nc.sync.dma_start(at8h_sb[:], at_d[0])
                    nc.sync.dma_start(at8l_sb[:], at_d[1])
                    nc.sync.dma_start(at8r_sb[:], at_d[2])
                elif kind in smalls:
                    t, d = smalls[kind]
                    nc.sync.dma_start(t[:], d[:])
                elif kind in ("r8", "f8"):
                    i = tok[1]
                    tl = xr8s[i] if kind == "r8" else xf8s[i]
                    dr = xr8_d if kind == "r8" else xf8_d
                    if len(tok) > 2:
                        lo, hi = tok[2], tok[3]
                        nc.sync.dma_start(tl[:, lo:hi, :], dr[i, :, lo:hi, :])
                    else:
                        nc.sync.dma_start(tl[:], dr[i])
                elif kind == "w":
                    dma_w(tok[1], tok[2], tok[3])
            for op in pe_seq:
                if op[0] == "d":
                    dummies(op[1])
                elif op[0] == "xa":
                    xa(op[1])
                elif op[0] == "tr":
                    tr(op[1])
                elif op[0] == "go":
                    group_open(op[1], op[2], op[3], op[4])
                elif op[0] == "gbc":
                    group_base_copy(op[1], op[2])
                elif op[0] == "ga":
                    group_adj(op[1], op[2])
                elif op[0] == "gc":
                    group_close(op[1], op[2])
                else:
                    group_open(op[1], op[2], 0, KP)
                    group_close(op[1], op[2])
    nc.compile()
    return nc


def _f8_pair(vals, scale):
    """Return (hi, lo) fp8 arrays s.t. hi+lo ~= vals*scale (all in fp8)."""
    v = vals.astype(np.float32) * scale
    hi = v.astype(NP_F8)
    lo = (v - hi.astype(np.float32)).astype(NP_F8)
    return hi, lo


def _prep_inputs(x, W, b, lora_A, lora_B):
    Wq = (W.astype(np.float32) * SW).astype(NP_F8)            # [out, in]
    # wt[ob, p, kp, t, c] = Wq[ob*512 + c, (2*kp + t)*128 + p]
    wt_in = np.ascontiguousarray(
        Wq.T.reshape(KP, 2, P, N_OBLK, O_BLK).transpose(3, 2, 0, 1, 4)
    )

    # at8[h][p, kp, t, r] = fp8 hi/lo pair of 4*A[r, (2kp+t)*128+p]
    a4 = (4.0 * lora_A.astype(np.float32).T) \
        .reshape(KP, 2, P, RANK).transpose(2, 0, 1, 3)
    at_hi = a4.astype(NP_F8)
    at_lo = (a4 - at_hi.astype(np.float32)).astype(NP_F8)
    at_r = (a4 / 8.0).astype(NP_F8)
    at_in = np.stack([at_hi, at_lo, at_r])                    # [3,P,KP,2,R]

    # bb[r, t, n]: partitions 0..15 and 16..31 both carry (B_hi, B_lo);
    # partition 32 carries (b_hi, b_lo) matched with the ones row of xaT8.
    bb_in = np.zeros((RP, 2, OUT_F), dtype=NP_F8)
    b_hi, b_lo = _f8_pair(lora_B.astype(np.float32).T, SB)    # [RANK, OUT_F]
    bb_in[0:RANK, 0, :] = b_hi
    bb_in[0:RANK, 1, :] = b_lo
    bb_in[RANK:2 * RANK, 0, :] = b_hi
    bb_in[RANK:2 * RANK, 1, :] = b_lo
    bias_hi, bias_lo = _f8_pair(b.astype(np.float32), PSC)
    bb_in[2 * RANK, 0, :] = bias_hi
    bb_in[2 * RANK, 1, :] = bias_lo

    id_in = np.eye(P, dtype=NP_BF)

    x_flat = np.ascontiguousarray(x.reshape(TOK, IN_F).astype(np.float32))
    in_maps = []
    for c in range(N_CORES):
        xc = x_flat[c * M_PER_CORE:(c + 1) * M_PER_CORE]      # [1024, 4096]
        # x[m, p, k, c] = xc[m*128 + c, k*128 + p]
        xt = np.ascontiguousarray(xc.T.reshape(KT, P, MT, P)
                                  .transpose(2, 1, 0, 3))
        xf8_in = np.ascontiguousarray(xt * SX).astype(NP_F8)
        xr = xt - xf8_in.astype(np.float32) / SX
        xr8_in = np.ascontiguousarray(xr * SR).astype(NP_F8)
        in_maps.append({
            "xf8": xf8_in,
            "xr8": xr8_in,
            "wt": wt_in,
            "at": at_in,
            "bb": bb_in,
            "ident": id_in,
        })
    return in_maps


def kernel(x, W, b, lora_A, lora_B, _trace=False):
    global LAST_RESULTS, _NC_CACHE

    in_maps = _prep_inputs(x, W, b, lora_A, lora_B)

    if _NC_CACHE is None:
        _NC_CACHE = _build_nc()
    nc = _NC_CACHE

    res = run_bass_kernel_spmd(nc, in_maps, core_ids=list(range(N_CORES)),
                               trace=_trace)
    LAST_RESULTS = res

    out = np.concatenate(
        [r["out"].reshape(M_PER_CORE, OUT_F).astype(np.float32)
         for r in res.results], axis=0)
    out *= np.float32(DESCALE)
    return out.reshape(B_SZ, S_SZ, OUT_F).astype(np.float32)


# revision 58
# speedup vs baseline: 1.0001x; 1.0001x over previous
"""LoRA Linear (x @ W.T + b + scaling * (x @ A.T) @ B.T) on 8 TRN2 NeuronCores.

Strategy (all-fp8 DoubleRow: base, adapter projection, and close):
  - Data-parallel: 8192 tokens -> 8 x 1024 rows, one shard per core.
  - Base matmul in fp8 e4m3 with MatmulPerfMode.DoubleRow (2 k-rows per
    partition, 2 moving rows/cycle -> 256 cycles per [128tok x 512out x 256k]
    instruction). 109.2us/core is the PE floor at 2.4 GHz.
  - x shipped as xf8 = fp8(8x) plus residual xr8 = fp8(512(x-xf8/8)); the
    base path uses xf8 alone (~1.4% rel err, dominated by fp8 noise on x
    and W); the adapter path uses both.
  - Adapter projection (per m-tile, all fp8 DoubleRow, two accumulators so
    no operand goes subnormal): psA = 8x (.) 4(A_hi+A_lo) = 32*x8@A over
    k-tile pairs; psB = 512xr (.) 4A_hi = 2048*xr@A_hi; combined on DVE as
    xa_c = psB/64 + psA = 32*xa (the dropped xr@A_lo term is ~0.2% of xa).
    Then an on-device fp8 hi/lo split (hi8 = fp8(xa_c/32), lo = xa - hi8),
    packed [128, 32], one PE transpose, copied twice into
    xaT8[33, 2, 1024] (partitions 0-15 hi rows, 16-31 lo rows, 32 = ones
    for the bias; t-dim duplicated).
  - Close: ONE fp8 DoubleRow matmul per group finishes each PSUM
    accumulation: psum += xaT8.T @ bb8 where bb8 pairs (B_hi, B_lo) per
    partition => (xa_hi+xa_lo)(B_hi+B_lo) = 2048*2*(xa@B.T), and partition
    32 adds 2048*b via its (b_hi, b_lo) pair. 256 cycles instead of the
    bf16 close's 512.
  - Scales: PSC = SX*SW = 8*256 = 2048 so bb8 = fp8(4096*B.T) stays under
    fp8e4 max 240. Host divides the output by 2048 (power of two, exact).
  - PE p-state (cost model): clock reaches 2.4 GHz only after ~3us of
    sustained execution and a >4us idle gap resets it -> 30 warmup dummies
    inside the initial DMA window, then no PE gap exceeds ~1.5us.
  - DMA: all inputs on the SP HWDGE queue, ordered so cumulative delivered
    bytes track PE consumption (x0/W0-quarters/x1 first, then x_i
    interleaved with w1..w3 halves, w4..w7 last, gated by the W pool's 5
    bufs). Outputs are buffered in SBUF and flushed one whole column per
    completion on the Activation queue - they never delay an input the PE
    is waiting for. Last column goes out per-m-tile on the by-then-idle SP
    queue to shorten the tail.
  - PE order: piecewise opens of (tiles 0-2, col0) as W0 quarters land with
    xa/transposes as filler, cols 1-2 for tiles 0-2 as w1/w2 halves land,
    then per-tile bands (i=3..7 across cols 0-2), then cols 3-7
    column-major. The cols 1-2 opens hide the xa->split->transpose->copy
    chain of each tile before its closes read xaT8.
"""

import numpy as np
import ml_dtypes

import concourse.bass as bass
from concourse import bacc
import concourse.mybir as mybir
import concourse.tile as tile
from concourse.bass_utils import run_bass_kernel_spmd

N_CORES = 8
IN_F = 4096
OUT_F = 4096
RANK = 16
ALPHA = 32.0
B_SZ = 4
S_SZ = 2048
TOK = B_SZ * S_SZ            # 8192
M_PER_CORE = TOK // N_CORES  # 1024

P = 128                      # partitions
KT = IN_F // P               # 32 k-tiles
KP = KT // 2                 # 16 k-pair tiles (DoubleRow)
O_BLK = 512                  # o-block width (psum bank = 512 fp32)
N_OBLK = OUT_F // O_BLK      # 8
MT = M_PER_CORE // P         # 8 m-tiles
RP = 2 * RANK + 1            # hi rows + lo rows + ones/bias row = 33

SX = 8.0                     # x fp8 scale
SR = 64.0                    # x residual fp8 scale (matches 32x psum
                             # scale through at8r = fp8(A/2))
SW = 256.0                   # W fp8 scale
PSC = SX * SW                # psum scale 2048
SB = PSC * (ALPHA / RANK)    # B scale in bb8 = 4096
DESCALE = 1.0 / PSC          # applied on host (power of two, exact)

F8 = mybir.dt.float8e4
BF = mybir.dt.bfloat16
F32 = mybir.dt.float32
NP_F8 = ml_dtypes.float8_e4m3
NP_BF = ml_dtypes.bfloat16

LAST_RESULTS = None          # test.py reads exec_time_ns from here
_NC_CACHE = None


def _default_schedule():
    # DMA tokens: ("at",)/("bb",)/("id",) consts; ("f8", i)/("r8", i) x
    # m-tiles; ("w", j, kp_lo, kp_hi) W o-block pieces.
    dma_seq = [
        ("f8", 0, 0, 16), ("f8", 0, 16, 32),
        ("w", 0, 0, 4), ("f8", 1), ("w", 0, 4, 8), ("w", 0, 8, 12),
        ("f8", 2), ("w", 0, 12, 16),
        ("w", 1, 0, 4), ("w", 1, 4, 8), ("w", 1, 8, 12), ("w", 1, 12, 16),
        ("w", 2, 0, 4), ("w", 2, 4, 8), ("w", 2, 8, 12), ("w", 2, 12, 16),
        ("f8", 3), ("r8", 3), ("id",), ("at",),
        ("r8", 0, 0, 16), ("r8", 0, 16, 32), ("bb",),
        ("f8", 4), ("r8", 4), ("r8", 1, 0, 16), ("r8", 1, 16, 32),
        ("f8", 5), ("r8", 5), ("r8", 2, 0, 16), ("r8", 2, 16, 32),
        ("w", 3, 0, 4), ("w", 3, 4, 8), ("f8", 6), ("r8", 6),
        ("w", 3, 8, 12), ("w", 3, 12, 16),
        ("f8", 7), ("r8", 7),
        ("w", 4, 0, 16), ("w", 5, 0, 16), ("w", 6, 0, 16), ("w", 7, 0, 16),
    ]
    # PE tokens: ("d", n) warmup dummies; ("go", i, j, kp_lo, kp_hi) partial
    # open (stop=True on kp 15 for decoupled groups); ("gbc", i, j) base-only
    # copy-out; ("ga", i, j) late adapter+bias patch (PE matmul + DVE add
    # into the SBUF column tile); ("gc", i, j) fused close; ("g", i, j)
    # full fused group; ("xa", i); ("tr", i).
    # ~30 dummies accumulate the p-state ramp during the DMA prefill.
    pe_seq = [("d", 30)]
    # phase A: tiles 0-2 over col0 as W0 quarters land, base-only (the
    # adapter inputs r8/at/bb are deferred behind w1/w2 on the wire)
    pe_seq += [("go", 0, 0, 0, 4),
               ("go", 1, 0, 0, 4),
               ("go", 0, 0, 4, 8), ("go", 1, 0, 4, 8),
               ("go", 0, 0, 8, 12), ("go", 1, 0, 8, 12),
               ("go", 2, 0, 0, 12),
               ("go", 0, 0, 12, 16), ("gbc", 0, 0),
               ("go", 1, 0, 12, 16), ("gbc", 1, 0),
               ("go", 2, 0, 12, 16), ("gbc", 2, 0)]
    # phase B: tiles 0-2 over cols 1-2 as w1/w2 halves land, all base-only
    pe_seq += [("go", 0, 1, 0, 8), ("go", 1, 1, 0, 8), ("go", 2, 1, 0, 8),
               ("go", 0, 1, 8, 16), ("gbc", 0, 1),
               ("go", 1, 1, 8, 16), ("gbc", 1, 1),
               ("go", 2, 1, 8, 16), ("gbc", 2, 1),
               ("go", 0, 2, 0, 8), ("go", 1, 2, 0, 8), ("go", 2, 2, 0, 8),
               ("go", 0, 2, 8, 16), ("gbc", 0, 2),
               ("go", 1, 2, 8, 16), ("gbc", 1, 2),
               ("go", 2, 2, 8, 16), ("gbc", 2, 2)]
    # phase C: tiles 3-7, each across cols 0-2 (fused closes); the xa/tr of
    # tiles 0-2 and all nine adapter patches ride the natural slack of these
    # blocks (every chain is hidden behind a 1.71us open)
    pe_seq += [("go", 3, 0, 0, 16), ("xa", 3), ("go", 3, 1, 0, 16),
               ("xa", 0), ("tr", 3), ("go", 3, 2, 0, 16), ("gc", 3, 0),
               ("tr", 0), ("gc", 3, 1), ("gc", 3, 2),
               ("ga", 0, 0), ("ga", 0, 1),
               ("go", 4, 0, 0, 16), ("xa", 4), ("go", 4, 1, 0, 16),
               ("xa", 1), ("tr", 4), ("go", 4, 2, 0, 16), ("gc", 4, 0),
               ("tr", 1), ("gc", 4, 1), ("gc", 4, 2),
               ("ga", 1, 0), ("ga", 1, 1), ("ga", 0, 2),
               ("go", 5, 0, 0, 16), ("xa", 5), ("go", 5, 1, 0, 16),
               ("xa", 2), ("tr", 5), ("go", 5, 2, 0, 16), ("gc", 5, 0),
               ("tr", 2), ("gc", 5, 1), ("gc", 5, 2),
               ("ga", 2, 0), ("ga", 2, 1), ("ga", 1, 2),
               ("go", 6, 0, 0, 16), ("xa", 6), ("go", 6, 1, 0, 16),
               ("tr", 6), ("go", 6, 2, 0, 16), ("gc", 6, 0),
               ("ga", 2, 2), ("gc", 6, 1), ("gc", 6, 2),
               ("go", 7, 0, 0, 16), ("xa", 7), ("go", 7, 1, 0, 16),
               ("tr", 7), ("go", 7, 2, 0, 16), ("gc", 7, 0),
               ("gc", 7, 1), ("gc", 7, 2)]
    # phase D: cols 3..7 column-major
    for j in range(3, N_OBLK):
        for i in range(MT):
            pe_seq.append(("g", i, j))
    return dma_seq, pe_seq


def _build_nc(schedule=None):
    dma_seq, pe_seq = schedule if schedule is not None else _default_schedule()

    nc = bacc.Bacc(None, target_bir_lowering=False)

    xf8_d = nc.dram_tensor("xf8", [MT, P, KT, P], F8, kind="ExternalInput")
    xr8_d = nc.dram_tensor("xr8", [MT, P, KT, P], F8, kind="ExternalInput")
    wt_d = nc.dram_tensor("wt", [N_OBLK, P, KP, 2, O_BLK], F8,
                          kind="ExternalInput")
    at_d = nc.dram_tensor("at", [3, P, KP, 2, RANK], F8,
                          kind="ExternalInput")
    bb_d = nc.dram_tensor("bb", [RP, 2, OUT_F], F8, kind="ExternalInput")
    id_d = nc.dram_tensor("ident", [P, P], BF, kind="ExternalInput")
    # [i][p][j][c] row-major == [1024, 4096] row-major
    out_d = nc.dram_tensor("out", [MT, P, N_OBLK, O_BLK], BF,
                           kind="ExternalOutput")

    with tile.TileContext(nc) as tc:
        with (
            tc.tile_pool(name="xp", bufs=1) as xp,
            tc.tile_pool(name="wp", bufs=5) as wp,
            tc.tile_pool(name="sm", bufs=1) as sm,
            tc.tile_pool(name="hip", bufs=2) as hip,
            tc.tile_pool(name="pkp", bufs=2) as pkp,
            tc.tile_pool(name="outs", bufs=4) as outs,
            tc.tile_pool(name="pmain", bufs=6, space="PSUM") as pmain,
            tc.tile_pool(name="pxa", bufs=1, space="PSUM") as pxa,
            tc.tile_pool(name="ptr", bufs=1, space="PSUM") as ptr,
        ):
            # ---- small constants ----
            at8h_sb = sm.tile([P, KP, 2, RANK], F8, tag="at8h")
            at8l_sb = sm.tile([P, KP, 2, RANK], F8, tag="at8l")
            at8r_sb = sm.tile([P, KP, 2, RANK], F8, tag="at8r")
            bb_sb = sm.tile([RP, 2, OUT_F], F8, tag="bb")
            id_sb = sm.tile([P, P], BF, tag="ident")
            warm = sm.tile([P, P], BF, tag="warm")
            nc.vector.memset(warm[:], 0.0)
            xaT8 = sm.tile([RP, 2, M_PER_CORE], F8, tag="xaT8")
            # partition 32 rows stay (1.0, 1.0): multiplied with bb's
            # (b_hi, b_lo) pair they add the bias. Partitions 0..31 are
            # overwritten column-range by column-range by the per-m-tile
            # transposed copies before any close reads them. (Walrus cannot
            # lower a memset at a nonzero partition offset, so set the whole
            # tile.)
            nc.vector.memset(xaT8[:], 1.0)
            smalls = {"bb": (bb_sb, bb_d), "id": (id_sb, id_d)}

            wps = pmain.tile([P, O_BLK], F32, tag="pm", name="warmps")

            xf8s, xr8s = [], []
            for i in range(MT):
                xf8s.append(xp.tile([P, KT, P], F8, tag=f"xf8_{i}",
                                    name=f"xf8_{i}"))
                xr8s.append(xp.tile([P, KT, P], F8, tag=f"xr8_{i}",
                                    name=f"xr8_{i}"))

            w_tiles = {}
            col_tiles = {}

            def dummies(n):
                for _ in range(n):
                    nc.tensor.matmul(wps[:, 0:P], warm[:], warm[:],
                                     start=True, stop=True)

            def dma_w(j, lo, hi):
                if j not in w_tiles:
                    w_tiles[j] = wp.tile([P, KP, 2, O_BLK], F8, tag="w",
                                         name=f"w{j}")
                t = w_tiles[j]
                if lo == 0 and hi == KP:
                    nc.sync.dma_start(t[:], wt_d[j])
                else:
                    nc.sync.dma_start(t[:, lo:hi, :, :],
                                      wt_d[j, :, lo:hi, :, :])

            xa_ps = {}

            def xa(i):
                # All-fp8 DoubleRow adapter projection, single accumulator
                # (operand scales chosen to dodge fp8 subnormals):
                #   ps = sum 8x*(4A_hi+4A_lo) + sum 64xr*(A/2) = 32 * xa
                ps = pxa.tile([P, RANK], F32, tag="pxa", name=f"pxa{i}")
                for kp in range(KP):
                    xp_sl = xf8s[i][:, 2 * kp:2 * kp + 2, :]
                    nc.tensor.matmul(
                        ps[:], xp_sl, at8h_sb[:, kp, :, :],
                        start=(kp == 0), stop=False,
                        perf_mode=mybir.MatmulPerfMode.DoubleRow,
                    )
                    nc.tensor.matmul(
                        ps[:], xp_sl, at8l_sb[:, kp, :, :],
                        start=False, stop=False,
                        perf_mode=mybir.MatmulPerfMode.DoubleRow,
                    )
                    nc.tensor.matmul(
                        ps[:], xr8s[i][:, 2 * kp:2 * kp + 2, :],
                        at8r_sb[:, kp, :, :],
                        start=False, stop=(kp == KP - 1),
                        perf_mode=mybir.MatmulPerfMode.DoubleRow,
                    )
                # fp8 hi/lo split: hi8 = fp8(xa); lo = xa - hi8 (bf16)
                hi8_t = hip.tile([P, RANK], F8, tag="hi8", name=f"hi8_{i}")
                nc.scalar.activation(hi8_t[:], ps[:],
                                     mybir.ActivationFunctionType.Copy,
                                     scale=1.0 / 32.0)
                pk = pkp.tile([P, 2, RANK], BF, tag="pack", name=f"pack{i}")
                nc.vector.tensor_copy(pk[:, 0, :], hi8_t[:])
                nc.vector.scalar_tensor_tensor(
                    pk[:, 1, :], ps[:], 1.0 / 32.0, pk[:, 0, :],
                    mybir.AluOpType.mult, mybir.AluOpType.subtract,
                )
                xa_ps[i] = pk

            def tr(i):
                pk = xa_ps.pop(i)
                trt = ptr.tile([2 * RANK, P], BF, tag="ptr", name=f"ptr{i}")
                nc.tensor.transpose(trt[:], pk[:], id_sb[:])
                cs = slice(i * P, (i + 1) * P)
                nc.scalar.copy(xaT8[0:2 * RANK, 0, cs], trt[:])
                nc.vector.tensor_copy(xaT8[0:2 * RANK, 1, cs], trt[:])

            gidx = 0
            col_remaining = {j: MT for j in range(N_OBLK)}
            open_ps = {}
            # groups whose base part is closed standalone (adapter patched
            # in later by ("ga", i, j))
            decoupled = {(i, j) for i in range(3) for j in range(3)}

            def group_open(i, j, lo, hi):
                if (i, j) in open_ps:
                    ps = open_ps[(i, j)]
                else:
                    ps = pmain.tile([P, O_BLK], F32, tag="pm",
                                    name=f"pm{i}_{j}")
                    open_ps[(i, j)] = ps
                for kp in range(lo, hi):
                    nc.tensor.matmul(
                        ps[:], xf8s[i][:, 2 * kp:2 * kp + 2, :],
                        w_tiles[j][:, kp, :, :],
                        start=(kp == 0),
                        stop=(kp == KP - 1 and (i, j) in decoupled),
                        perf_mode=mybir.MatmulPerfMode.DoubleRow,
                    )

            def ensure_col(j):
                if j not in col_tiles:
                    col_tiles[j] = outs.tile([P, MT, O_BLK], BF, tag="col",
                                             name=f"col{j}")

            def group_base_copy(i, j):
                nonlocal gidx
                ps = open_ps.pop((i, j))
                ensure_col(j)
                if gidx % 2 == 0:
                    nc.scalar.copy(col_tiles[j][:, i, :], ps[:])
                else:
                    nc.vector.tensor_copy(col_tiles[j][:, i, :], ps[:])
                gidx += 1

            def group_adj(i, j):
                # late adapter+bias patch: adj = xaT8.T @ bb8 in a fresh
                # psum, then col_tile += adj on DVE
                ps = pmain.tile([P, O_BLK], F32, tag="pm",
                                name=f"adj{i}_{j}")
                nc.tensor.matmul(
                    ps[:], xaT8[:, :, i * P:(i + 1) * P],
                    bb_sb[:, :, j * O_BLK:(j + 1) * O_BLK],
                    start=True, stop=True,
                    perf_mode=mybir.MatmulPerfMode.DoubleRow,
                )
                nc.vector.scalar_tensor_tensor(
                    col_tiles[j][:, i, :], ps[:], 1.0,
                    col_tiles[j][:, i, :],
                    mybir.AluOpType.mult, mybir.AluOpType.add,
                )
                col_remaining[j] -= 1

            def group_close(i, j):
                nonlocal gidx
                ps = open_ps.pop((i, j))
                ensure_col(j)
                # fp8 DoubleRow close: (xa_hi+xa_lo)(B_hi+B_lo) + bias row
                nc.tensor.matmul(
                    ps[:], xaT8[:, :, i * P:(i + 1) * P],
                    bb_sb[:, :, j * O_BLK:(j + 1) * O_BLK],
                    start=False, stop=True,
                    perf_mode=mybir.MatmulPerfMode.DoubleRow,
                )
                # Output DMAs are deferred to column completion so they never
                # delay input transfers the PE is still waiting for. Last
                # column goes out per-m-tile in half-width pieces (copies
                # split across ACT/DVE, DMAs on the by-then-idle SP queue) to
                # shorten the end-of-kernel tail.
                if j == N_OBLK - 1:
                    # per-m-tile pieces on the by-then-idle SP queue keep the
                    # end-of-kernel tail short
                    if gidx % 2 == 0:
                        nc.scalar.copy(col_tiles[j][:, i, :], ps[:])
                    else:
                        nc.vector.tensor_copy(col_tiles[j][:, i, :], ps[:])
                    nc.sync.dma_start(
                        out_d[i:i + 1, :, j, :].rearrange("i p c -> p i c"),
                        col_tiles[j][:, i:i + 1, :],
                    )
                else:
                    if gidx % 2 == 0:
                        nc.scalar.copy(col_tiles[j][:, i, :], ps[:])
                    else:
                        nc.vector.tensor_copy(col_tiles[j][:, i, :], ps[:])
                gidx += 1
                col_remaining[j] -= 1
                done = MT - col_remaining[j]
                if j < N_OBLK - 1 and done == MT:
                    nc.scalar.dma_start(
                        out_d[:, :, j, :].rearrange("i p c -> p i c"),
                        col_tiles[j][:],
                    )

            for tok in dma_seq:
                kind = tok[0]
                if kind == "at":
                    nc.sync.dma_start(at8h_sb[:], at_d[0])
                    nc.sync.dma_start(at8l_sb[:], at_d[1])
                    nc.sync.dma_start(at8r_sb[:], at_d[2])
                elif kind in smalls:
                    t, d = smalls[kind]
                    nc.sync.dma_start(t[:], d[:])
                elif kind in ("r8", "f8"):
                    i = tok[1]
                    tl = xr8s[i] if kind == "r8" else xf8s[i]
                    dr = xr8_d if kind == "r8" else xf8_d
                    if len(tok) > 2:
                        lo, hi = tok[2], tok[3]
                        nc.sync.dma_start(tl[:, lo:hi, :], dr[i, :, lo:hi, :])
                    else:
                        nc.sync.dma_start(tl[:], dr[i])
                elif kind == "w":
                    dma_w(tok[1], tok[2], tok[3])
            for op in pe_seq:
                if op[0] == "d":
                    dummies(op[1])
                elif op[0] == "xa":
                    xa(op[1])
                elif op[0] == "tr":
                    tr(op[1])
                elif op[0] == "go":
                    group_open(op[1], op[2], op[3], op[4])
                elif op[0] == "gbc":
                    group_base_copy(op[1], op[2])
                elif op[0] == "ga":
                    group_adj(op[1], op[2])
                elif op[0] == "gc":
                    group_close(op[1], op[2])
                else:
                    group_open(op[1], op[2], 0, KP)
                    group_close(op[1], op[2])
    nc.compile()
    return nc


def _f8_pair(vals, scale):
    """Return (hi, lo) fp8 arrays s.t. hi+lo ~= vals*scale (all in fp8)."""
    v = vals.astype(np.float32) * scale
    hi = v.astype(NP_F8)
    lo = (v - hi.astype(np.float32)).astype(NP_F8)
    return hi, lo


def _prep_inputs(x, W, b, lora_A, lora_B):
    Wq = (W.astype(np.float32) * SW).astype(NP_F8)            # [out, in]
    # wt[ob, p, kp, t, c] = Wq[ob*512 + c, (2*kp + t)*128 + p]
    wt_in = np.ascontiguousarray(
        Wq.T.reshape(KP, 2, P, N_OBLK, O_BLK).transpose(3, 2, 0, 1, 4)
    )

    # at8[h][p, kp, t, r] = fp8 hi/lo pair of 4*A[r, (2kp+t)*128+p]
    a4 = (4.0 * lora_A.astype(np.float32).T) \
        .reshape(KP, 2, P, RANK).transpose(2, 0, 1, 3)
    at_hi = a4.astype(NP_F8)
    at_lo = (a4 - at_hi.astype(np.float32)).astype(NP_F8)
    at_r = (a4 / 8.0).astype(NP_F8)
    at_in = np.stack([at_hi, at_lo, at_r])                    # [3,P,KP,2,R]

    # bb[r, t, n]: partitions 0..15 and 16..31 both carry (B_hi, B_lo);
    # partition 32 carries (b_hi, b_lo) matched with the ones row of xaT8.
    bb_in = np.zeros((RP, 2, OUT_F), dtype=NP_F8)
    b_hi, b_lo = _f8_pair(lora_B.astype(np.float32).T, SB)    # [RANK, OUT_F]
    bb_in[0:RANK, 0, :] = b_hi
    bb_in[0:RANK, 1, :] = b_lo
    bb_in[RANK:2 * RANK, 0, :] = b_hi
    bb_in[RANK:2 * RANK, 1, :] = b_lo
    bias_hi, bias_lo = _f8_pair(b.astype(np.float32), PSC)
    bb_in[2 * RANK, 0, :] = bias_hi
    bb_in[2 * RANK, 1, :] = bias_lo

    id_in = np.eye(P, dtype=NP_BF)

    x_flat = np.ascontiguousarray(x.reshape(TOK, IN_F).astype(np.float32))
    in_maps = []
    for c in range(N_CORES):
        xc = x_flat[c * M_PER_CORE:(c + 1) * M_PER_CORE]      # [1024, 4096]
        # x[m, p, k, c] = xc[m*128 + c, k*128 + p]
        xt = np.ascontiguousarray(xc.T.reshape(KT, P, MT, P)
                                  .transpose(2, 1, 0, 3))
        xf8_in = np.ascontiguousarray(xt * SX).astype(NP_F8)
        xr = xt - xf8_in.astype(np.float32) / SX
        xr8_in = np.ascontiguousarray(xr * SR).astype(NP_F8)
        in_maps.append({
            "xf8": xf8_in,
            "xr8": xr8_in,
            "wt": wt_in,
            "at": at_in,
            "bb": bb_in,
            "ident": id_in,
        })
    return in_maps


def kernel(x, W, b, lora_A, lora_B, _trace=False):
    global LAST_RESULTS, _NC_CACHE

    in_maps = _prep_inputs(x, W, b, lora_A, lora_B)

    if _NC_CACHE is None:
        _NC_CACHE = _build_nc()
    nc = _NC_CACHE

    res = run_bass_kernel_spmd(nc, in_maps, core_ids=list(range(N_CORES)),
                               trace=_trace)
    LAST_RESULTS = res

    out = np.concatenate(
        [r["out"].reshape(M_PER_CORE, OUT_F).astype(np.float32)
         for r in res.results], axis=0)
    out *= np.float32(DESCALE)
    return out.reshape(B_SZ, S_SZ, OUT_F).astype(np.float32)
